# revision 19
# baseline (speedup 1.0000x reference)
"""Trainium2 Bass kernel for nn_DiscAdvLossForTarget_min.

Math (per batch row, x = logits[0:1000], e = extra logit x[1000]):
    prob_i = softmax(x)_i                  = exp(x_i - e) / sum_j exp(x_j - e)
    log pc_i = log sigmoid(e - x_i)        = -log1p(exp(x_i - e))
    loss = -(1/B) * sum_b sum_i prob_i * log(pc_i)
         = +(1/B) * sum_b U_b / S_b
    where a_i = exp(x_i - e), U_b = sum_i a_i * log1p(a_i), S_b = sum_i a_i.

Device mapping (per core, data-parallel over batch, 8192 rows per core).
Per supertile of G=8 row-blocks, the S-row-sum work is split between the
two row-sum-capable engines so neither is the bottleneck:
  mode A (blocks 0..KA-1): ACT a = Exp(x + bias(-e)), accum_out -> S col
  mode D (blocks KA..G-1): one batched ACT Exp -> t; DVE tensor_scalar
      a = t * exp(-e) with accum_out -> S col (1x reduce path)
  then one batched ACT Ln(a + 1) -> w (same table set as Exp), and a DVE
  scalar_tensor_tensor (a * 1) * w with accum_out -> U col per block.
Intermediates a/t/w are bf16 (halves SBUF; accumulators stay fp32).
Host: loss = (1/B) * sum over rows/cores of U/S.
"""

import numpy as np

import bass_rust as _bass_rust
import concourse.bacc as bacc
import concourse.bass as bass
import concourse.tile as tile
from concourse import bass_utils, mybir
from concourse.hw_specs import get_activation_tables

N_CORES = 8
B_FULL = 65536
C1 = 1001
C = 1000
P = 128
B_SHARD = B_FULL // N_CORES  # 8192
G = 8  # batch row-blocks per supertile (one DMA each)
KA = 3  # blocks whose S-reduce rides the ACT Exp accumulator
KD = G - KA
N_ST = B_SHARD // (P * G)  # 8
N_BLOCKS = B_SHARD // P  # 64


class _PinnedBacc(bacc.Bacc):
    """Bacc whose activation-table chooser only sees sets containing every
    activation function this kernel uses, so Exp and Ln resolve to one
    resident set (natural_log_exp_and_others) instead of thrashing
    ACT_TABLE_LOADs between per-function sets."""

    def insert_act_table_loads(self):
        used = {
            i.func
            for b in self.main_func.blocks
            for i in b.instructions
            if isinstance(i, mybir.InstActivation)
        }
        if not used:
            return
        tables = [
            (name, fns if used <= fns else set())
            for name, fns in get_activation_tables(self.m.arch).items()
        ]
        _bass_rust.insert_act_table_loads(self, tables)


_nc_cache = None


def _build() -> bass.Bass:
    global _nc_cache
    if _nc_cache is not None:
        return _nc_cache

    nc = _PinnedBacc("TRN2", debug=False)
    x = nc.dram_tensor("x", [B_SHARD, C1], mybir.dt.float32, kind="ExternalInput").ap()
    u_out = nc.dram_tensor(
        "u_out", [P, N_BLOCKS], mybir.dt.float32, kind="ExternalOutput"
    ).ap()
    s_out = nc.dram_tensor(
        "s_out", [P, N_BLOCKS], mybir.dt.float32, kind="ExternalOutput"
    ).ap()

    # Shard row handled by (partition p, supertile s, slot g): row = (p*N_ST + s)*G + g.
    # Per partition, one supertile's G rows are contiguous in DRAM (32 KiB).
    x_r = x.rearrange("(p s g) m -> s p g m", p=P, s=N_ST, g=G)

    with tile.TileContext(nc) as tc:
        with (
            tc.tile_pool(name="xin", bufs=2) as xin,
            tc.tile_pool(name="mid", bufs=2) as mid,
            tc.tile_pool(name="mid3", bufs=3) as mid3,
            tc.tile_pool(name="small", bufs=3) as small,
            tc.tile_pool(name="accp", bufs=1) as accp,
        ):
            U = accp.tile([P, N_BLOCKS], mybir.dt.float32)
            S = accp.tile([P, N_BLOCKS], mybir.dt.float32)
            for s in range(N_ST):
                xt = xin.tile([P, G, C1], mybir.dt.float32, tag="xt")
                if s == 0:
                    # split the pipeline-fill load so compute starts after
                    # the first block lands rather than the whole 4 MiB
                    for g in range(G):
                        nc.sync.dma_start(out=xt[:, g, :], in_=x_r[s][:, g, :])
                else:
                    nc.sync.dma_start(out=xt, in_=x_r[s])

                aa = mid3.tile([P, G, C], mybir.dt.bfloat16, tag="aa")

                # mode A: Exp with per-partition bias(-e) + accumulator row-sum
                neg_e = small.tile([P, KA], mybir.dt.float32, tag="neg_e")
                nc.scalar.mul(neg_e, xt[:, 0:KA, C], -1.0)
                for g in range(KA):
                    col = s * G + g
                    nc.scalar.activation(
                        out=aa[:, g, :],
                        in_=xt[:, g, 0:C],
                        func=mybir.ActivationFunctionType.Exp,
                        bias=neg_e[:, g : g + 1],
                        scale=1.0,
                        accum_out=S[:, col : col + 1],
                    )

                # mode D: batched Exp (incl. e columns), then DVE scale+reduce
                tt = mid.tile([P, KD, C1], mybir.dt.bfloat16, tag="tt")
                nc.scalar.activation(
                    out=tt.rearrange("p g c -> p (g c)"),
                    in_=xt[:, KA:G, :].rearrange("p g c -> p (g c)"),
                    func=mybir.ActivationFunctionType.Exp,
                )
                cc = small.tile([P, KD], mybir.dt.float32, tag="cc")
                nc.vector.reciprocal(cc, tt[:, :, C])
                for j in range(KD):
                    col = s * G + KA + j
                    nc.vector.tensor_scalar(
                        out=aa[:, KA + j, :],
                        in0=tt[:, j, 0:C],
                        scalar1=cc[:, j : j + 1],
                        scalar2=0.0,
                        op0=mybir.AluOpType.mult,
                        op1=mybir.AluOpType.add,
                        accum_out=S[:, col : col + 1],
                    )

                ww = mid3.tile([P, G, C], mybir.dt.bfloat16, tag="ww")
                nc.scalar.activation(
                    out=ww.rearrange("p g c -> p (g c)"),
                    in_=aa.rearrange("p g c -> p (g c)"),
                    func=mybir.ActivationFunctionType.Ln,
                    bias=1.0,
                    scale=1.0,
                )

                for g in range(G):
                    col = s * G + g
                    scr = mid.tile([P, C], mybir.dt.bfloat16, tag="scr")
                    nc.vector.scalar_tensor_tensor(
                        out=scr,
                        in0=aa[:, g, :],
                        scalar=1.0,
                        in1=ww[:, g, :],
                        op0=mybir.AluOpType.mult,
                        op1=mybir.AluOpType.mult,
                        accum_out=U[:, col : col + 1],
                    )

            nc.sync.dma_start(out=u_out, in_=U)
            nc.sync.dma_start(out=s_out, in_=S)

    nc.finalize()  # runs Bacc passes (wait splitting, reg alloc, ...)
    _nc_cache = nc
    return nc


LAST_RESULTS = None


def kernel(input: np.ndarray, target: np.ndarray | None = None, _trace: bool = False, **_unused) -> np.ndarray:
    global LAST_RESULTS
    input = np.ascontiguousarray(np.asarray(input, dtype=np.float32))
    assert input.shape == (B_FULL, C1), input.shape

    nc = _build()
    in_maps = [
        {"x": input[i * B_SHARD : (i + 1) * B_SHARD]} for i in range(N_CORES)
    ]
    res = bass_utils.run_bass_kernel_spmd(
        nc, in_maps, core_ids=list(range(N_CORES)), trace=_trace
    )
    LAST_RESULTS = res
    total = np.float64(0.0)
    for r in res.results:
        u = np.asarray(r["u_out"], dtype=np.float64)
        s = np.asarray(r["s_out"], dtype=np.float64)
        total += (u / s).sum()
    # w = log1p(a) = -log(pc) already carries the loss's minus sign.
    loss = total / B_FULL
    return np.float32(loss)
